# revision 18
# baseline (speedup 1.0000x reference)
"""Causal single-head attention (B=4, S=2048, D=1024) on 8 NeuronCores.

Sharding: core c owns the q rows {2i + (c%2)} of batch c//2 (1024 rows).
Interleaving q rows by parity gives every core an identical causal
block structure, so one SPMD program serves all 8 cores; only the data
(and the staircase mask) differs per core.

Key order is globally redefined as [parity-0 rows asc, parity-1 rows
asc] — attention is invariant to key permutation as long as K, V and
the mask agree. Under that order each core's q rows are its own parity
half, its causal extent per q-block j is the uniform tile set
[0, 4(j+1)) + [8, 8+4(j+1)) (128-key tiles), and exactly 8 tiles per
block cross the diagonal. Crossing tile with in-block offset c is
fully masked on its first 128*c q columns: scores/exp run only on the
remaining columns (left part memset to 0) and AV matmuls for
q-subtiles u < c are skipped. The staircase mask depends only on the
crossing offset (128c + r <= i has no j term), so ONE [P, 8, QB] mask
serves both q-blocks.

K/V projections are deduplicated across the core pair of each batch:
core p computes K/V only for its parity rows, and the pair exchanges
halves with 2-core AllGathers (DRAM bounce), chunked so the collectives
and read-backs pipeline under the q projection and score matmuls.

Softmax denominators ride the AV loop as N=1 matmuls (w.T @ ones)
that reuse the AV matmuls' stationary operand; the denominator matmul
of the last tile is issued BEFORE that tile's AV matmuls so the
reciprocal overlaps the group's tail instead of extending it.

DMA/overlap notes: each dma_start costs ~0.6us on its trigger queue
(Sync or Scalar; Vector cannot trigger), so the ramp-critical inputs
are batched into few large transfers and split across BOTH trigger
queues: xo in 4 chunks of 2 d-tiles on Sync (arrival tracks the
d-outermost consumption order of stage A), wk in 4 E-column chunks on
Scalar (chunk c feeds wave c). All host-side layouts are pre-tiled so
every input DMA is a contiguous 2D transfer. The remaining 5MB
(wq/wv in 2 chunks each, mask, ones) is deferred (add_dep_helper)
behind stage A's first reduction to give the ramp-critical 4.5MB full
bandwidth. Dummy matmuls on a zeroed tile warm the PE clock (HAM)
during the initial DMA wait (36 is enough to bridge to the first xo/wk
arrival), and stage A runs waves of 4 concurrent PSUM groups
(d-outermost, borrowing the idle av-pool banks) so the PE advances as
each input tile lands. Output is written bf16 (host upcasts) to halve
the output-DMA tail.
"""

import sys
import types

import numpy as np
import ml_dtypes

import concourse.tile as tile
from concourse import bacc, mybir
from concourse.bass_utils import run_bass_kernel_spmd


def _ensure_ntff_hook():
    """bass_utils imports antenv.axon_hooks when tracing; some containers
    lack that module. Register a process-local equivalent so trace=True
    works (or degrades to untraced instead of crashing)."""
    try:
        import antenv.axon_hooks  # noqa: F401
        return
    except ImportError:
        pass
    hook = None
    try:
        from trn_agent_boot.trn_boot import _ntff_profile_via_ctypes
        hook = _ntff_profile_via_ctypes("/opt/axon/libaxon_pjrt.so")
    except Exception:
        hook = None
    mod = types.ModuleType("antenv.axon_hooks")
    mod.get_axon_ntff_profile_hook = lambda: hook
    mod.set_axon_ntff_profile_hook = lambda h: None
    sys.modules["antenv.axon_hooks"] = mod


_ensure_ntff_hook()

BF16 = mybir.dt.bfloat16
F32 = mybir.dt.float32
AF = mybir.ActivationFunctionType

B, S, D = 4, 2048, 1024
P = 128
NCORES = 8
SQ = 1024            # q rows per core (= own parity half)
ND = D // P          # 8 contraction tiles over d
NE = D // P          # 8 tiles over e (d_out)
NSK = S // P         # 16 key tiles
QB = 512             # q-block width (matmul free dim)
NQB = SQ // QB       # 2 q blocks
SCALE = 1.0 / np.sqrt(np.float32(D))
PAIRS = [[2 * b, 2 * b + 1] for b in range(B)]

TRACE = False
LAST_RESULT = None

_cache = {}


def _sk_list(j):
    # key tiles needed by q-block j: prefix of each parity half
    return list(range(0, 4 * (j + 1))) + list(range(8, 8 + 4 * (j + 1)))


def _cross_list(j):
    # diagonal-crossing key tiles of q-block j (order matches maskd)
    return list(range(4 * j, 4 * (j + 1))) + list(range(8 + 4 * j, 8 + 4 * (j + 1)))


def _coff(j, t):
    # in-block crossing offset: first 128*c q columns of tile t are fully
    # masked within q-block j (c = 0 for non-crossing computed tiles)
    return max(0, (t % 8) - 4 * j)


def _build():
    nc = bacc.Bacc("TRN2", target_bir_lowering=False, debug=False,
                   num_devices=NCORES)
    # all inputs host-pre-tiled so each chunk is one contiguous 2D DMA
    xot = nc.dram_tensor("xot", [4, P, 2, SQ], BF16, kind="ExternalInput")
    wkt = nc.dram_tensor("wkt", [4, P, ND, 2 * P], BF16, kind="ExternalInput")
    wvt = nc.dram_tensor("wvt", [2, P, 4, D], BF16, kind="ExternalInput")
    wqt = nc.dram_tensor("wqt", [2, P, 4, D], BF16, kind="ExternalInput")
    maskd = nc.dram_tensor("maskd", [P, 8, QB], BF16, kind="ExternalInput")
    ones = nc.dram_tensor("ones", [P, 8], BF16, kind="ExternalInput")
    out = nc.dram_tensor("out", [SQ, D], BF16, kind="ExternalOutput")

    from contextlib import ExitStack
    with tile.TileContext(nc) as tc:
        with ExitStack() as ctx:
            xo_pool = ctx.enter_context(tc.tile_pool(name="xo", bufs=4))
            wk_pool = ctx.enter_context(tc.tile_pool(name="wk", bufs=4))
            wv_pool = ctx.enter_context(tc.tile_pool(name="wv", bufs=2))
            wq_pool = ctx.enter_context(tc.tile_pool(name="wq", bufs=2))
            st_pool = ctx.enter_context(tc.tile_pool(name="st", bufs=6))
            kT_pool = ctx.enter_context(tc.tile_pool(name="kT", bufs=1))
            v_pool = ctx.enter_context(tc.tile_pool(name="v", bufs=1))
            qT_pool = ctx.enter_context(tc.tile_pool(name="qT", bufs=NE))
            m_pool = ctx.enter_context(tc.tile_pool(name="mk", bufs=1))
            we_pool = ctx.enter_context(tc.tile_pool(name="we", bufs=24))
            on_pool = ctx.enter_context(tc.tile_pool(name="on", bufs=2))
            sm_pool = ctx.enter_context(tc.tile_pool(name="sm", bufs=2))
            rc_pool = ctx.enter_context(tc.tile_pool(name="rc", bufs=4))
            o_pool = ctx.enter_context(tc.tile_pool(name="o", bufs=2))
            dr_pool = ctx.enter_context(
                tc.tile_pool(name="dr", bufs=10, space="DRAM"))
            ps_pool = ctx.enter_context(
                tc.tile_pool(name="ps", bufs=3, space="PSUM"))
            av_pool = ctx.enter_context(
                tc.tile_pool(name="av", bufs=2, space="PSUM"))
            rs_pool = ctx.enter_context(
                tc.tile_pool(name="rs", bufs=1, space="PSUM"))
            # ---- input DMAs ----
            # ramp-critical (stage A): xo + wk chunks interleaved on Sync.
            # Everything stays on the Sync trigger queue (hardware queue
            # Q1): scalar-triggered DMAs land on Q10, which the collective
            # engine needs — stealing it serializes the AllGather stream.
            xo_c = [xo_pool.tile([P, 2, SQ], BF16, tag="xo", name=f"xoc{c}")
                    for c in range(4)]
            wk_c = [wk_pool.tile([P, ND, 2 * P], BF16, tag="wk",
                                 name=f"wkc{c}")
                    for c in range(4)]
            for c in range(4):
                nc.sync.dma_start(xo_c[c][:], xot[c])
                nc.sync.dma_start(wk_c[c][:], wkt[c])

            def xo_s(d, sl):     # xo[d] slice [P, sl]
                return xo_c[d // 2][:, d % 2, sl]

            def wk_s(d, e):      # wk[d][:, e*P:(e+1)*P]
                return wk_c[e // 2][:, d, (e % 2) * P:(e % 2) * P + P]

            # later-stage inputs are gated behind stage A's first psum
            # group (see below) so the ramp-critical transfers get the
            # full DMA bandwidth
            # deferred bulk on the Scalar trigger queue: the Sync queue must
            # stay free of long-waiting triggers so exchange writes and
            # output DMAs fire the moment their data is ready
            wv_c, wq_c = [], []
            deferred = []
            for c in range(2):
                t = wv_pool.tile([P, 4, D], BF16, tag="wv")
                deferred.append(nc.scalar.dma_start(t[:], wvt[c]))
                wv_c.append(t)
            for c in range(2):
                t = wq_pool.tile([P, 4, D], BF16, tag="wq")
                deferred.append(nc.scalar.dma_start(t[:], wqt[c]))
                wq_c.append(t)
            mask_big = m_pool.tile([P, 8, QB], BF16, tag="mk")
            deferred.append(nc.scalar.dma_start(mask_big[:], maskd[:]))
            ones_t = on_pool.tile([P, 8], BF16, tag="on")
            deferred.append(nc.scalar.dma_start(ones_t[:], ones[:]))

            def wv_s(d, sl):
                return wv_c[d // 4][:, d % 4, sl]

            def wq_s(d, sl):
                return wq_c[d // 4][:, d % 4, sl]

            kT_big = kT_pool.tile([P, NE, S], BF16, tag="kT")
            v_big = v_pool.tile([P, NSK, D], BF16, tag="v")

            warm = st_pool.tile([P, P], BF16, tag="warm")
            nc.vector.memset(warm[:], 0.0)
            wps = ps_pool.tile([P, P], F32, tag="ps")
            for i in range(46):
                nc.tensor.matmul(wps[:], warm[:], warm[:],
                                 start=(i == 0), stop=(i == 45))

            # tiny dummy collective fired immediately: the CC stream opens
            # with an all-core BARRIER costing ~29us — ring it early so the
            # first real AllGather isn't paying it (output is never read)
            cc_warm_sb = st_pool.tile([P, 8], BF16, tag="ccw")
            nc.vector.memset(cc_warm_sb[:], 0.0)
            cc_warm_dr = dr_pool.tile([P, 8], BF16, tag="ccwi")
            cc_warm_out = dr_pool.tile([2, P, 8], BF16, tag="ccwo")
            nc.sync.dma_start(cc_warm_dr[:], cc_warm_sb[:])
            nc.gpsimd.collective_compute(
                "AllGather", mybir.AluOpType.bypass, replica_groups=PAIRS,
                ins=[cc_warm_dr.opt()], outs=[cc_warm_out.opt()],
            )

            # ---- stage A: kT own half [e, s0], exchanged in 2 KEY-half
            # chunks (scores j=0 consumes only key-half A of every E, so
            # stage D starts before the second AllGather lands) ----
            exk0_dmas = []
            ag_k = []
            for kh in range(2):
                ex_in = dr_pool.tile([8, P, QB], BF16, tag=f"exik{kh}",
                                     name=f"exik{kh}")
                ex_out = dr_pool.tile([2, 8, P, QB], BF16, tag=f"exok{kh}",
                                      name=f"exok{kh}")
                # waves of 4 concurrent psum groups (2 ps + 2 borrowed
                # av-pool banks), d-outermost: during the input-DMA ramp the
                # PE advances every open group as each d tile lands instead
                # of stalling on one group's full reduction
                for Eh in range(2):
                    kst = st_pool.tile([P, 4, QB], BF16, tag="st",
                                       name=f"kst{kh}{Eh}")
                    pss = [ps_pool.tile([P, QB], F32, tag="ps",
                                        name=f"aps{kh}{Eh}{g}")
                           if g < 2 else
                           av_pool.tile([P, QB], F32, tag="av",
                                        name=f"aav{kh}{Eh}{g}")
                           for g in range(4)]
                    for d in range(ND):
                        for g in range(4):
                            E = 4 * Eh + g
                            nc.tensor.matmul(
                                pss[g][:],
                                wk_s(d, E),
                                xo_s(d, slice(kh * QB, (kh + 1) * QB)),
                                start=(d == 0), stop=(d == ND - 1),
                            )
                    for g in range(4):
                        nc.vector.tensor_copy(kst[:, g, :], pss[g][:])
                    dm = nc.sync.dma_start(
                        ex_in[4 * Eh:4 * Eh + 4].rearrange("n p m -> p n m"),
                        kst[:])
                    if kh == 0:
                        exk0_dmas.append(dm)
                if kh == 0:
                    # release the bulk loads only once the K-A exchange
                    # writes have full bandwidth — their completion rings
                    # the first real collective doorbell
                    from concourse.bass import _add_dep_helper
                    for dd in deferred:
                        for dm in exk0_dmas:
                            _add_dep_helper(
                                dd.ins, dm.ins, sync=True,
                                reason="defer bulk loads past K-A exchange")
                nc.gpsimd.collective_compute(
                    "AllGather", mybir.AluOpType.bypass, replica_groups=PAIRS,
                    ins=[ex_in.opt()], outs=[ex_out.opt()],
                )
                ag_k.append(ex_out)

            def k_readback(kh):
                # on Scalar: a readback trigger waits for its AllGather, so
                # it must never sit ahead of time-critical Sync writes
                ex_out = ag_k[kh]
                for r in range(2):
                    nc.scalar.dma_start(
                        kT_big[:, :, r * SQ + kh * QB:r * SQ + (kh + 1) * QB],
                        ex_out[r].rearrange("n p m -> p n m"))

            k_readback(0)

            # ---- stage B: v own half [s0, e], exchanged in 2 s-chunks.
            # h=1 is emitted AFTER stage C: the Q projection is 27us of
            # pure compute with no DMA traffic, which gives the K
            # AllGathers + readbacks (and V-h0) a quiet HBM window.
            # Readbacks are emitted separately at a point where their
            # AG-completion wait cannot block earlier Sync writes ----
            ag_v = []

            def stage_b(h):
                ex_in = dr_pool.tile([4, P, D], BF16, tag=f"exiv{h}",
                                     name=f"exiv{h}")
                ex_out = dr_pool.tile([2, 4, P, D], BF16, tag=f"exov{h}",
                                      name=f"exov{h}")
                for so in range(4):
                    sT = 4 * h + so
                    vst = st_pool.tile([P, D], BF16, tag="st",
                                       name=f"vst{h}{so}")
                    for ec in range(D // QB):
                        ps = ps_pool.tile([P, QB], F32, tag="ps",
                                          name=f"bps{h}{so}{ec}")
                        for d in range(ND):
                            nc.tensor.matmul(
                                ps[:],
                                xo_s(d, slice(sT * P, (sT + 1) * P)),
                                wv_s(d, slice(ec * QB, (ec + 1) * QB)),
                                start=(d == 0), stop=(d == ND - 1),
                            )
                        nc.vector.tensor_copy(
                            vst[:, ec * QB:(ec + 1) * QB], ps[:])
                    nc.sync.dma_start(ex_in[so], vst[:])
                nc.gpsimd.collective_compute(
                    "AllGather", mybir.AluOpType.bypass, replica_groups=PAIRS,
                    ins=[ex_in.opt()], outs=[ex_out.opt()],
                )
                ag_v.append(ex_out)

            def v_readback(h):
                ex_out = ag_v[h]
                for r in range(2):
                    for i in range(2):
                        nc.sync.dma_start(
                            v_big[:, 8 * r + 4 * h + 2 * i:
                                  8 * r + 4 * h + 2 * (i + 1), :],
                            ex_out[r, 2 * i:2 * (i + 1)].rearrange(
                                "n p m -> p n m"))

            stage_b(0)

            # ---- stage C: qT[e, i] from own rows ----
            qT_t = []
            for E in range(NE):
                t = qT_pool.tile([P, SQ], BF16, tag="qT")
                qT_t.append(t)
            for E in range(NE):
                for qc in range(SQ // QB):
                    ps = ps_pool.tile([P, QB], F32, tag="ps")
                    for d in range(ND):
                        nc.tensor.matmul(
                            ps[:],
                            wq_s(d, slice(E * P, (E + 1) * P)),
                            xo_s(d, slice(qc * QB, (qc + 1) * QB)),
                            start=(d == 0), stop=(d == ND - 1),
                        )
                    nc.vector.tensor_copy(qT_t[E][:, qc * QB:(qc + 1) * QB], ps[:])

            stage_b(1)
            v_readback(0)

            # ---- stage D: attention. Scores for BOTH q-blocks are emitted
            # before any AV pass so the PE consumption order matches the
            # AllGather delivery order (kT-A, kT-B, v-A, v-B); within j=1
            # the key-half-A tiles run first to tolerate a late kT-B ----
            wtiles = {}
            for j in range(NQB):
                if j == 1:
                    k_readback(1)
                cross = _cross_list(j)
                sk = sorted(_sk_list(j), key=lambda t: ((t % 8) >= 4, t))
                for t in sk:
                    c = _coff(j, t)
                    w0 = c * P          # first live q column of this tile
                    ps = ps_pool.tile([P, QB], F32, tag="ps")
                    for E in range(NE):
                        nc.tensor.matmul(
                            ps[:, 0:QB - w0],
                            kT_big[:, E, t * P:(t + 1) * P],
                            qT_t[E][:, j * QB + w0:(j + 1) * QB],
                            start=(E == 0), stop=(E == NE - 1),
                        )
                    wt = we_pool.tile([P, QB], BF16, tag="we")
                    nc.scalar.activation(wt[:, w0:QB], ps[:, 0:QB - w0],
                                         AF.Exp, scale=float(SCALE))
                    if t in cross:
                        tt = cross.index(t)
                        nc.vector.tensor_mul(wt[:, w0:QB], wt[:, w0:QB],
                                             mask_big[:, tt, w0:QB])
                    wtiles[(j, t)] = wt

            for j in range(NQB):
                if j == 1:
                    v_readback(1)
                for u in range(QB // P):
                    ts_u = sorted(
                        (t for t in _sk_list(j) if _coff(j, t) <= u),
                        key=lambda t: ((t % 8) >= 4, t))
                    av = av_pool.tile([P, D], F32, tag="av")
                    rs = rs_pool.tile([P, 1], F32, tag="rs")
                    n = len(ts_u)
                    for idx, t in enumerate(ts_u):
                        lhsT = wtiles[(j, t)][:, u * P:(u + 1) * P]
                        st, sp = idx == 0, idx == n - 1
                        nc.tensor.matmul(rs[:], lhsT, ones_t[:, 0:1],
                                         start=st, stop=sp)
                        nc.tensor.matmul(av[:, 0:QB], lhsT, v_big[:, t, 0:QB],
                                         start=st, stop=sp)
                        nc.tensor.matmul(av[:, QB:D], lhsT, v_big[:, t, QB:D],
                                         start=st, stop=sp)
                    rcp = rc_pool.tile([P, 1], F32, tag="rcp")
                    nc.vector.reciprocal(rcp[:], rs[:])
                    ot = o_pool.tile([P, D], BF16, tag="o")
                    r0 = (j * (QB // P) + u) * P
                    # out-scale on ScalarE (per-partition scale AP) keeps
                    # Vector free for the mask muls and shortens the
                    # AV-group boundary chain
                    for eh in range(2):
                        nc.scalar.activation(
                            ot[:, eh * QB:(eh + 1) * QB],
                            av[:, eh * QB:(eh + 1) * QB],
                            AF.Copy, scale=rcp[:])
                        nc.sync.dma_start(out[r0:r0 + P, eh * QB:(eh + 1) * QB],
                                          ot[:, eh * QB:(eh + 1) * QB])

    nc.compile()
    return nc


def _prep_inputs(x, Wq, Wk, Wv):
    bf = ml_dtypes.bfloat16

    def dtile(a):     # [D, n] -> [P, ND, n] (partition-major d-tiles)
        return a.reshape(ND, P, a.shape[1]).transpose(1, 0, 2)

    # wk: 4 E-column chunks [4, P, ND, 2P], each contiguous
    wk_b = np.ascontiguousarray(
        dtile(Wk).reshape(P, ND, 4, 2 * P).transpose(2, 0, 1, 3).astype(bf))
    # wv/wq: 2 chunks of 4 d-tiles [2, P, 4, D]
    wv_b = np.ascontiguousarray(
        dtile(Wv).reshape(P, 2, 4, D).transpose(1, 0, 2, 3).astype(bf))
    wq_b = np.ascontiguousarray(
        dtile(Wq).reshape(P, 2, 4, D).transpose(1, 0, 2, 3).astype(bf))
    ones = np.ones((P, 8), bf)
    ks = np.arange(S)
    ii = np.arange(SQ)
    # global index of permuted key position (parity-0 rows, then parity-1)
    gk = np.where(ks < SQ, 2 * ks, 2 * (ks - SQ) + 1)
    in_maps = []
    for c in range(NCORES):
        b, p = c // 2, c % 2
        xoT = x[b, p::2].T                          # [D, SQ]
        # xo: 4 chunks of 2 d-tiles [4, P, 2, SQ], each contiguous
        xo_b = np.ascontiguousarray(
            dtile(xoT).reshape(P, 4, 2, SQ).transpose(1, 0, 2, 3).astype(bf))
        gq = 2 * ii + p
        # staircase mask is q-block independent: build from block j=0
        maskd = np.zeros((8, P, QB), np.float32)
        for tt, t in enumerate(_cross_list(0)):
            gk_t = gk[t * P:(t + 1) * P]
            maskd[tt] = (gk_t[:, None] <= gq[None, :QB]).astype(np.float32)
        mask_dev = np.ascontiguousarray(
            maskd.transpose(1, 0, 2).astype(bf))    # [P, 8, QB]
        in_maps.append({
            "xot": xo_b, "wqt": wq_b, "wkt": wk_b, "wvt": wv_b,
            "maskd": mask_dev, "ones": ones,
        })
    return in_maps


def kernel(x, Wq, Wk, Wv):
    global LAST_RESULT
    x = np.asarray(x, np.float32)
    Wq = np.asarray(Wq, np.float32)
    Wk = np.asarray(Wk, np.float32)
    Wv = np.asarray(Wv, np.float32)

    if "nc" not in _cache:
        _cache["nc"] = _build()
    nc = _cache["nc"]

    in_maps = _prep_inputs(x, Wq, Wk, Wv)
    res = run_bass_kernel_spmd(nc, in_maps, list(range(NCORES)), trace=TRACE)
    LAST_RESULT = res

    out = np.empty((B, S, D), np.float32)
    for c in range(NCORES):
        b, p = c // 2, c % 2
        out[b, p::2, :] = res.results[c]["out"].astype(np.float32)
    return out


# revision 21
# speedup vs baseline: 1.0664x; 1.0664x over previous
"""Causal single-head attention (B=4, S=2048, D=1024) on 8 NeuronCores.

Sharding: core c owns the q rows {2i + (c%2)} of batch c//2 (1024 rows).
Interleaving q rows by parity gives every core an identical causal
block structure, so one SPMD program serves all 8 cores; only the data
(and the staircase mask) differs per core.

Key order is globally redefined as [parity-0 rows asc, parity-1 rows
asc] — attention is invariant to key permutation as long as K, V and
the mask agree. Under that order each core's q rows are its own parity
half, its causal extent per q-block j is the uniform tile set
[0, 4(j+1)) + [8, 8+4(j+1)) (128-key tiles), and exactly 8 tiles per
block cross the diagonal. Crossing tile with in-block offset c is
fully masked on its first 128*c q columns: scores/exp run only on the
remaining columns (left part memset to 0) and AV matmuls for
q-subtiles u < c are skipped. The staircase mask depends only on the
crossing offset (128c + r <= i has no j term), so ONE [P, 8, QB] mask
serves both q-blocks.

K/V projections are deduplicated across the core pair of each batch:
core p computes K/V only for its parity rows, and the pair exchanges
halves with 2-core AllGathers (DRAM bounce), chunked so the collectives
and read-backs pipeline under the q projection and score matmuls.

Softmax denominators ride the AV loop as N=1 matmuls (w.T @ ones)
that reuse the AV matmuls' stationary operand; the denominator matmul
of the last tile is issued BEFORE that tile's AV matmuls so the
reciprocal overlaps the group's tail instead of extending it.

DMA/overlap notes: each dma_start costs ~0.6us on its trigger queue
(Sync or Scalar; Vector cannot trigger), so the ramp-critical inputs
are batched into few large transfers and split across BOTH trigger
queues: xo in 4 chunks of 2 d-tiles on Sync (arrival tracks the
d-outermost consumption order of stage A), wk in 4 E-column chunks on
Scalar (chunk c feeds wave c). All host-side layouts are pre-tiled so
every input DMA is a contiguous 2D transfer. The remaining 5MB
(wq/wv in 2 chunks each, mask, ones) is deferred (add_dep_helper)
behind stage A's first reduction to give the ramp-critical 4.5MB full
bandwidth. Dummy matmuls on a zeroed tile warm the PE clock (HAM)
during the initial DMA wait (36 is enough to bridge to the first xo/wk
arrival), and stage A runs waves of 4 concurrent PSUM groups
(d-outermost, borrowing the idle av-pool banks) so the PE advances as
each input tile lands. Output is written bf16 (host upcasts) to halve
the output-DMA tail.
"""

import sys
import types

import numpy as np
import ml_dtypes

import concourse.tile as tile
from concourse import bacc, mybir
from concourse.bass_utils import run_bass_kernel_spmd


def _ensure_ntff_hook():
    """bass_utils imports antenv.axon_hooks when tracing; some containers
    lack that module. Register a process-local equivalent so trace=True
    works (or degrades to untraced instead of crashing)."""
    try:
        import antenv.axon_hooks  # noqa: F401
        return
    except ImportError:
        pass
    hook = None
    try:
        from trn_agent_boot.trn_boot import _ntff_profile_via_ctypes
        hook = _ntff_profile_via_ctypes("/opt/axon/libaxon_pjrt.so")
    except Exception:
        hook = None
    mod = types.ModuleType("antenv.axon_hooks")
    mod.get_axon_ntff_profile_hook = lambda: hook
    mod.set_axon_ntff_profile_hook = lambda h: None
    sys.modules["antenv.axon_hooks"] = mod


_ensure_ntff_hook()

BF16 = mybir.dt.bfloat16
F32 = mybir.dt.float32
AF = mybir.ActivationFunctionType

B, S, D = 4, 2048, 1024
P = 128
NCORES = 8
SQ = 1024            # q rows per core (= own parity half)
ND = D // P          # 8 contraction tiles over d
NE = D // P          # 8 tiles over e (d_out)
NSK = S // P         # 16 key tiles
QB = 512             # q-block width (matmul free dim)
NQB = SQ // QB       # 2 q blocks
SCALE = 1.0 / np.sqrt(np.float32(D))
PAIRS = [[2 * b, 2 * b + 1] for b in range(B)]

TRACE = False
LAST_RESULT = None

_cache = {}


def _sk_list(j):
    # key tiles needed by q-block j: prefix of each parity half
    return list(range(0, 4 * (j + 1))) + list(range(8, 8 + 4 * (j + 1)))


def _cross_list(j):
    # diagonal-crossing key tiles of q-block j (order matches maskd)
    return list(range(4 * j, 4 * (j + 1))) + list(range(8 + 4 * j, 8 + 4 * (j + 1)))


def _coff(j, t):
    # in-block crossing offset: first 128*c q columns of tile t are fully
    # masked within q-block j (c = 0 for non-crossing computed tiles)
    return max(0, (t % 8) - 4 * j)


def _build():
    nc = bacc.Bacc("TRN2", target_bir_lowering=False, debug=False,
                   num_devices=NCORES)
    # all inputs host-pre-tiled so each chunk is one contiguous 2D DMA
    xot = nc.dram_tensor("xot", [4, P, 2, SQ], BF16, kind="ExternalInput")
    wkt = nc.dram_tensor("wkt", [4, P, ND, 2 * P], BF16, kind="ExternalInput")
    wvt = nc.dram_tensor("wvt", [2, P, 4, D], BF16, kind="ExternalInput")
    wqt = nc.dram_tensor("wqt", [2, P, 4, D], BF16, kind="ExternalInput")
    maskd = nc.dram_tensor("maskd", [P, 8, QB], BF16, kind="ExternalInput")
    ones = nc.dram_tensor("ones", [P, 8], BF16, kind="ExternalInput")
    out = nc.dram_tensor("out", [SQ, D], BF16, kind="ExternalOutput")

    from contextlib import ExitStack
    with tile.TileContext(nc) as tc:
        with ExitStack() as ctx:
            xo_pool = ctx.enter_context(tc.tile_pool(name="xo", bufs=4))
            wk_pool = ctx.enter_context(tc.tile_pool(name="wk", bufs=4))
            wv_pool = ctx.enter_context(tc.tile_pool(name="wv", bufs=2))
            wq_pool = ctx.enter_context(tc.tile_pool(name="wq", bufs=2))
            st_pool = ctx.enter_context(tc.tile_pool(name="st", bufs=6))
            kT_pool = ctx.enter_context(tc.tile_pool(name="kT", bufs=1))
            v_pool = ctx.enter_context(tc.tile_pool(name="v", bufs=1))
            qT_pool = ctx.enter_context(tc.tile_pool(name="qT", bufs=NE))
            m_pool = ctx.enter_context(tc.tile_pool(name="mk", bufs=1))
            we_pool = ctx.enter_context(tc.tile_pool(name="we", bufs=24))
            on_pool = ctx.enter_context(tc.tile_pool(name="on", bufs=2))
            sm_pool = ctx.enter_context(tc.tile_pool(name="sm", bufs=2))
            rc_pool = ctx.enter_context(tc.tile_pool(name="rc", bufs=4))
            o_pool = ctx.enter_context(tc.tile_pool(name="o", bufs=2))
            dr_pool = ctx.enter_context(
                tc.tile_pool(name="dr", bufs=10, space="DRAM"))
            ps_pool = ctx.enter_context(
                tc.tile_pool(name="ps", bufs=3, space="PSUM"))
            av_pool = ctx.enter_context(
                tc.tile_pool(name="av", bufs=2, space="PSUM"))
            rs_pool = ctx.enter_context(
                tc.tile_pool(name="rs", bufs=1, space="PSUM"))
            # ---- input DMAs ----
            # ramp-critical (stage A): xo + wk chunks interleaved on Sync.
            # Everything stays on the Sync trigger queue (hardware queue
            # Q1): scalar-triggered DMAs land on Q10, which the collective
            # engine needs — stealing it serializes the AllGather stream.
            xo_c = [xo_pool.tile([P, 2, SQ], BF16, tag="xo", name=f"xoc{c}")
                    for c in range(4)]
            wk_c = [wk_pool.tile([P, ND, 2 * P], BF16, tag="wk",
                                 name=f"wkc{c}")
                    for c in range(4)]
            for c in range(4):
                nc.sync.dma_start(xo_c[c][:], xot[c])
                nc.sync.dma_start(wk_c[c][:], wkt[c])

            def xo_s(d, sl):     # xo[d] slice [P, sl]
                return xo_c[d // 2][:, d % 2, sl]

            def wk_s(d, e):      # wk[d][:, e*P:(e+1)*P]
                return wk_c[e // 2][:, d, (e % 2) * P:(e % 2) * P + P]

            # later-stage inputs are gated behind stage A's first psum
            # group (see below) so the ramp-critical transfers get the
            # full DMA bandwidth
            # deferred bulk on the Scalar trigger queue: the Sync queue must
            # stay free of long-waiting triggers so exchange writes and
            # output DMAs fire the moment their data is ready
            wv_c, wq_c = [], []
            deferred = []
            for c in range(2):
                t = wv_pool.tile([P, 4, D], BF16, tag="wv")
                deferred.append(nc.scalar.dma_start(t[:], wvt[c]))
                wv_c.append(t)
            for c in range(2):
                t = wq_pool.tile([P, 4, D], BF16, tag="wq")
                deferred.append(nc.scalar.dma_start(t[:], wqt[c]))
                wq_c.append(t)
            mask_big = m_pool.tile([P, 8, QB], BF16, tag="mk")
            deferred.append(nc.scalar.dma_start(mask_big[:], maskd[:]))
            ones_t = on_pool.tile([P, 8], BF16, tag="on")
            deferred.append(nc.scalar.dma_start(ones_t[:], ones[:]))

            def wv_s(d, sl):
                return wv_c[d // 4][:, d % 4, sl]

            def wq_s(d, sl):
                return wq_c[d // 4][:, d % 4, sl]

            kT_big = kT_pool.tile([P, NE, S], BF16, tag="kT")
            v_big = v_pool.tile([P, NSK, D], BF16, tag="v")

            warm = st_pool.tile([P, P], BF16, tag="warm")
            nc.vector.memset(warm[:], 0.0)
            wps = ps_pool.tile([P, P], F32, tag="ps")
            for i in range(46):
                nc.tensor.matmul(wps[:], warm[:], warm[:],
                                 start=(i == 0), stop=(i == 45))

            # ---- stage A: kT own half [e, s0], exchanged in 2 KEY-half
            # chunks (scores j=0 consumes only key-half A of every E, so
            # stage D starts before the second AllGather lands) ----
            exk0_dmas = []
            ag_k = []
            for kh in range(2):
                ex_in = dr_pool.tile([8, P, QB], BF16, tag=f"exik{kh}",
                                     name=f"exik{kh}")
                ex_out = dr_pool.tile([2, 8, P, QB], BF16, tag=f"exok{kh}",
                                      name=f"exok{kh}")
                # waves of 4 concurrent psum groups (2 ps + 2 borrowed
                # av-pool banks), d-outermost: during the input-DMA ramp the
                # PE advances every open group as each d tile lands instead
                # of stalling on one group's full reduction
                for Eh in range(2):
                    kst = st_pool.tile([P, 4, QB], BF16, tag="st",
                                       name=f"kst{kh}{Eh}")
                    pss = [ps_pool.tile([P, QB], F32, tag="ps",
                                        name=f"aps{kh}{Eh}{g}")
                           if g < 2 else
                           av_pool.tile([P, QB], F32, tag="av",
                                        name=f"aav{kh}{Eh}{g}")
                           for g in range(4)]
                    for d in range(ND):
                        for g in range(4):
                            E = 4 * Eh + g
                            nc.tensor.matmul(
                                pss[g][:],
                                wk_s(d, E),
                                xo_s(d, slice(kh * QB, (kh + 1) * QB)),
                                start=(d == 0), stop=(d == ND - 1),
                            )
                    for g in range(4):
                        nc.vector.tensor_copy(kst[:, g, :], pss[g][:])
                    dm = nc.sync.dma_start(
                        ex_in[4 * Eh:4 * Eh + 4].rearrange("n p m -> p n m"),
                        kst[:])
                    if kh == 0:
                        exk0_dmas.append(dm)
                if kh == 0:
                    # release the bulk loads only once the K-A exchange
                    # writes have full bandwidth — their completion rings
                    # the first real collective doorbell
                    from concourse.bass import _add_dep_helper
                    for dd in deferred:
                        for dm in exk0_dmas:
                            _add_dep_helper(
                                dd.ins, dm.ins, sync=True,
                                reason="defer bulk loads past K-A exchange")
                nc.gpsimd.collective_compute(
                    "AllGather", mybir.AluOpType.bypass, replica_groups=PAIRS,
                    ins=[ex_in.opt()], outs=[ex_out.opt()],
                )
                ag_k.append(ex_out)

            def k_readback(kh):
                # emitted at a point where this trigger's AllGather-wait
                # resolves no later than the Sync writes queued behind it
                ex_out = ag_k[kh]
                for r in range(2):
                    nc.sync.dma_start(
                        kT_big[:, :, r * SQ + kh * QB:r * SQ + (kh + 1) * QB],
                        ex_out[r].rearrange("n p m -> p n m"))

            # ---- stage B: v own half [s0, e], exchanged in 2 s-chunks.
            # h=1 is emitted AFTER stage C: the Q projection is 27us of
            # pure compute with no DMA traffic, which gives the K
            # AllGathers + readbacks (and V-h0) a quiet HBM window.
            # Readbacks are emitted separately at a point where their
            # AG-completion wait cannot block earlier Sync writes ----
            ag_v = []

            def stage_b(h):
                ex_in = dr_pool.tile([4, P, D], BF16, tag=f"exiv{h}",
                                     name=f"exiv{h}")
                ex_out = dr_pool.tile([2, 4, P, D], BF16, tag=f"exov{h}",
                                      name=f"exov{h}")
                for so in range(4):
                    sT = 4 * h + so
                    vst = st_pool.tile([P, D], BF16, tag="st",
                                       name=f"vst{h}{so}")
                    for ec in range(D // QB):
                        ps = ps_pool.tile([P, QB], F32, tag="ps",
                                          name=f"bps{h}{so}{ec}")
                        for d in range(ND):
                            nc.tensor.matmul(
                                ps[:],
                                xo_s(d, slice(sT * P, (sT + 1) * P)),
                                wv_s(d, slice(ec * QB, (ec + 1) * QB)),
                                start=(d == 0), stop=(d == ND - 1),
                            )
                        nc.vector.tensor_copy(
                            vst[:, ec * QB:(ec + 1) * QB], ps[:])
                    nc.sync.dma_start(ex_in[so], vst[:])
                nc.gpsimd.collective_compute(
                    "AllGather", mybir.AluOpType.bypass, replica_groups=PAIRS,
                    ins=[ex_in.opt()], outs=[ex_out.opt()],
                )
                ag_v.append(ex_out)

            def v_readback(h):
                ex_out = ag_v[h]
                for r in range(2):
                    for i in range(2):
                        nc.sync.dma_start(
                            v_big[:, 8 * r + 4 * h + 2 * i:
                                  8 * r + 4 * h + 2 * (i + 1), :],
                            ex_out[r, 2 * i:2 * (i + 1)].rearrange(
                                "n p m -> p n m"))

            stage_b(0)
            k_readback(0)
            stage_b(1)
            k_readback(1)
            v_readback(0)

            # ---- stage C: qT[e, i] from own rows ----
            qT_t = []
            for E in range(NE):
                t = qT_pool.tile([P, SQ], BF16, tag="qT")
                qT_t.append(t)
            for E in range(NE):
                for qc in range(SQ // QB):
                    ps = ps_pool.tile([P, QB], F32, tag="ps")
                    for d in range(ND):
                        nc.tensor.matmul(
                            ps[:],
                            wq_s(d, slice(E * P, (E + 1) * P)),
                            xo_s(d, slice(qc * QB, (qc + 1) * QB)),
                            start=(d == 0), stop=(d == ND - 1),
                        )
                    nc.vector.tensor_copy(qT_t[E][:, qc * QB:(qc + 1) * QB], ps[:])

            v_readback(1)

            # ---- stage D: attention per q block ----
            for j in range(NQB):
                sk_list = _sk_list(j)
                cross = _cross_list(j)
                wtiles = {}
                for t in sk_list:
                    c = _coff(j, t)
                    w0 = c * P          # first live q column of this tile
                    ps = ps_pool.tile([P, QB], F32, tag="ps")
                    for E in range(NE):
                        nc.tensor.matmul(
                            ps[:, 0:QB - w0],
                            kT_big[:, E, t * P:(t + 1) * P],
                            qT_t[E][:, j * QB + w0:(j + 1) * QB],
                            start=(E == 0), stop=(E == NE - 1),
                        )
                    wt = we_pool.tile([P, QB], BF16, tag="we")
                    nc.scalar.activation(wt[:, w0:QB], ps[:, 0:QB - w0],
                                         AF.Exp, scale=float(SCALE))
                    if t in cross:
                        tt = cross.index(t)
                        nc.vector.tensor_mul(wt[:, w0:QB], wt[:, w0:QB],
                                             mask_big[:, tt, w0:QB])
                    wtiles[t] = wt

                for u in range(QB // P):
                    ts_u = sorted(
                        (t for t in sk_list if _coff(j, t) <= u),
                        key=lambda t: ((t % 8) >= 4, t))
                    av = av_pool.tile([P, D], F32, tag="av")
                    rs = rs_pool.tile([P, 1], F32, tag="rs")
                    n = len(ts_u)
                    for idx, t in enumerate(ts_u):
                        lhsT = wtiles[t][:, u * P:(u + 1) * P]
                        st, sp = idx == 0, idx == n - 1
                        # denominator group closes before the last AV pair
                        # so the reciprocal overlaps the group's tail
                        nc.tensor.matmul(rs[:], lhsT, ones_t[:, 0:1],
                                         start=st, stop=sp)
                        nc.tensor.matmul(av[:, 0:QB], lhsT, v_big[:, t, 0:QB],
                                         start=st, stop=sp)
                        nc.tensor.matmul(av[:, QB:D], lhsT, v_big[:, t, QB:D],
                                         start=st, stop=sp)
                    rcp = rc_pool.tile([P, 1], F32, tag="rcp")
                    nc.vector.reciprocal(rcp[:], rs[:])
                    ot = o_pool.tile([P, D], BF16, tag="o")
                    r0 = (j * (QB // P) + u) * P
                    # split the two out-scales across ScalarE and Vector so
                    # they run in parallel at the AV-group boundary
                    nc.scalar.activation(ot[:, 0:QB], av[:, 0:QB],
                                         AF.Copy, scale=rcp[:])
                    nc.sync.dma_start(out[r0:r0 + P, 0:QB], ot[:, 0:QB])
                    nc.vector.tensor_scalar_mul(ot[:, QB:D], av[:, QB:D],
                                                rcp[:])
                    nc.sync.dma_start(out[r0:r0 + P, QB:D], ot[:, QB:D])

    nc.compile()
    return nc


def _prep_inputs(x, Wq, Wk, Wv):
    bf = ml_dtypes.bfloat16

    def dtile(a):     # [D, n] -> [P, ND, n] (partition-major d-tiles)
        return a.reshape(ND, P, a.shape[1]).transpose(1, 0, 2)

    # wk: 4 E-column chunks [4, P, ND, 2P], each contiguous
    wk_b = np.ascontiguousarray(
        dtile(Wk).reshape(P, ND, 4, 2 * P).transpose(2, 0, 1, 3).astype(bf))
    # wv/wq: 2 chunks of 4 d-tiles [2, P, 4, D]
    wv_b = np.ascontiguousarray(
        dtile(Wv).reshape(P, 2, 4, D).transpose(1, 0, 2, 3).astype(bf))
    wq_b = np.ascontiguousarray(
        dtile(Wq).reshape(P, 2, 4, D).transpose(1, 0, 2, 3).astype(bf))
    ones = np.ones((P, 8), bf)
    ks = np.arange(S)
    ii = np.arange(SQ)
    # global index of permuted key position (parity-0 rows, then parity-1)
    gk = np.where(ks < SQ, 2 * ks, 2 * (ks - SQ) + 1)
    in_maps = []
    for c in range(NCORES):
        b, p = c // 2, c % 2
        xoT = x[b, p::2].T                          # [D, SQ]
        # xo: 4 chunks of 2 d-tiles [4, P, 2, SQ], each contiguous
        xo_b = np.ascontiguousarray(
            dtile(xoT).reshape(P, 4, 2, SQ).transpose(1, 0, 2, 3).astype(bf))
        gq = 2 * ii + p
        # staircase mask is q-block independent: build from block j=0
        maskd = np.zeros((8, P, QB), np.float32)
        for tt, t in enumerate(_cross_list(0)):
            gk_t = gk[t * P:(t + 1) * P]
            maskd[tt] = (gk_t[:, None] <= gq[None, :QB]).astype(np.float32)
        mask_dev = np.ascontiguousarray(
            maskd.transpose(1, 0, 2).astype(bf))    # [P, 8, QB]
        in_maps.append({
            "xot": xo_b, "wqt": wq_b, "wkt": wk_b, "wvt": wv_b,
            "maskd": mask_dev, "ones": ones,
        })
    return in_maps


def kernel(x, Wq, Wk, Wv):
    global LAST_RESULT
    x = np.asarray(x, np.float32)
    Wq = np.asarray(Wq, np.float32)
    Wk = np.asarray(Wk, np.float32)
    Wv = np.asarray(Wv, np.float32)

    if "nc" not in _cache:
        _cache["nc"] = _build()
    nc = _cache["nc"]

    in_maps = _prep_inputs(x, Wq, Wk, Wv)
    res = run_bass_kernel_spmd(nc, in_maps, list(range(NCORES)), trace=TRACE)
    LAST_RESULT = res

    out = np.empty((B, S, D), np.float32)
    for c in range(NCORES):
        b, p = c // 2, c % 2
        out[b, p::2, :] = res.results[c]["out"].astype(np.float32)
    return out
